# revision 21
# baseline (speedup 1.0000x reference)
"""Sparse-attention transformer block on 8 Trainium2 NeuronCores (Bass/Tile).

Sharding: 8 cores = 4 batches x 2 query-halves (SPMD, one program).
Each core processes T=1024 query tokens of one batch. Key/value tokens are
host-gathered per core: the core's own 1024 tokens plus the (static-mask)
summary tokens its queries attend outside that range, padded to SKV=1408.
All activations are feature-major ("transposed", [feature, token]) so every
matmul contracts along partitions with zero on-device transposes:

  xT -> LN1 (partition reduce via ones-matmul) -> hT
  kT = Wk hT, qT = Wq hT (feature-major); V = hT^T Wv^T (token-major)
  scoresT[s,q] = kT^T qT per head; p = exp(s) * mask01 (scores are small:
  no max subtraction needed); oT[d,q] = V'^T p with a ones column in V'
  giving the softmax denominator for free; normalize, Wo, residual, LN2,
  MLP (gelu-tanh), residual -> outT.

Matmuls run in bf16 (tolerance 2e-2 >> bf16 error).
"""

import numpy as np
import ml_dtypes
from contextlib import ExitStack

import concourse.bass as bass
import concourse.bacc as bacc
import concourse.tile as tile
from concourse import mybir
from concourse import bass_utils

B, S, E, H, D = 4, 2048, 1024, 16, 64
HID = 4 * E
T = 1024            # query tokens per core
SKV = 1408          # gathered kv tokens per core (11 chunks of 128)
NCH = SKV // 128    # 11 s-chunks
NE = E // 128       # 8 feature chunks
NHT = HID // 128    # 32 hidden chunks
N_CORES = 8
# Active s-chunk pairs per query group. For queries q < 512 (qg0), keys are
# causally <= 511 (chunks 0-3) plus gathered summary chunks 8-10; chunks 4-7
# are fully masked for every core (host asserts this).
PAIRS_QG = [
    [(0, 1), (2, 3), (8, 9), (10,)],
    [(0, 1), (2, 3), (4, 5), (6, 7), (8, 9), (10,)],
]
MASK_J0 = [0, len(PAIRS_QG[0])]           # mask tile base index per qg
N_MASK = len(PAIRS_QG[0]) + len(PAIRS_QG[1])
MASK_W = 1024
QGS = [slice(0, 512), slice(512, 1024)]
KVGS = [slice(0, 512), slice(512, 1024), slice(1024, 1408)]
BF = mybir.dt.bfloat16
F32 = mybir.dt.float32
AF = mybir.ActivationFunctionType
OP = mybir.AluOpType

_CACHE = {}
import os
SIM_SAFE_GELU = os.environ.get("BASS_SIM_SAFE_GELU") == "1"


# ---------------------------------------------------------------- device code

def _tln(tc, ctx, src, dst, g_ap, b_ap, W, groups, ones, eps_row, tag,
         ones_f32=None):
    """Transposed layernorm: src [128, NE, W] bf16 -> dst [128, NE, W] bf16.

    Stats are partition reductions via ones-matmuls; mean/rstd rows get
    partition-broadcast via rank-1 PE matmuls (ones_f32 outer product) into
    PSUM -- no DRAM bounce. g_ap/b_ap: [128, NE]."""
    nc = tc.nc
    tmp = ctx.enter_context(tc.tile_pool(name=f"tmp{tag}", bufs=2))
    rows = ctx.enter_context(tc.tile_pool(name=f"rows{tag}", bufs=1))
    bc = ctx.enter_context(tc.tile_pool(name=f"bc{tag}", bufs=1, space="PSUM"))

    # stats rows live in partition 0 of the (later) broadcast tiles: the
    # row chain consumes them before the rank-1 matmuls overwrite the banks
    mean_b = bc.tile([128, W], F32, tag="mean_b")
    rstd_b = bc.tile([128, W], F32, tag="rstd_b")
    ps_mean = mean_b[0:1, :]
    ps_sq = rstd_b[0:1, :]
    for gs in groups:
        for c in range(NE):
            nc.tensor.matmul(ps_mean[:, gs], ones[:], src[:, c, gs],
                             start=(c == 0), stop=(c == NE - 1),
                             skip_group_check=True)
    for c in range(NE):
        sq = tmp.tile([128, W], BF, tag="sq")
        nc.scalar.activation(sq[:], src[:, c, :], AF.Square)
        for gs in groups:
            nc.tensor.matmul(ps_sq[:, gs], ones[:], sq[:, gs],
                             start=(c == 0), stop=(c == NE - 1),
                             skip_group_check=True)

    _tln_finish(tc, ctx, src, dst, g_ap, b_ap, W, ones_f32, mean_b, rstd_b,
                eps_row, tag)


def _tln_finish(tc, ctx, src, dst, g_ap, b_ap, W, ones_f32, mean_b, rstd_b,
                eps_row, tag):
    """LN tail: raw sums sit in partition 0 of mean_b/rstd_b (PSUM).
    Computes mean/rstd rows, rank-1 partition-broadcasts them back into
    the same banks, then normalizes src -> dst."""
    nc = tc.nc
    tmp = ctx.enter_context(tc.tile_pool(name=f"tmpf{tag}", bufs=2))
    rows = ctx.enter_context(tc.tile_pool(name=f"rows{tag}", bufs=1))
    ps_mean = mean_b[0:1, :]
    ps_sq = rstd_b[0:1, :]

    # rows chain with two recycled slots (A: mean, B: scratch)
    mean_r = rows.tile([1, W], F32, tag="rowA")
    nc.scalar.mul(mean_r[:], ps_mean[:, :], 1.0 / E)
    msq_r = rows.tile([1, W], F32, tag="rowB")
    nc.vector.tensor_mul(msq_r[:], mean_r[:], mean_r[:])
    var_r = rows.tile([1, W], F32, tag="rowC")
    nc.vector.scalar_tensor_tensor(var_r[:], ps_sq[:, :], 1.0 / E, msq_r[:],
                                   op0=OP.mult, op1=OP.subtract)
    std_r = rows.tile([1, W], F32, tag="rowB")
    nc.scalar.activation(std_r[:], var_r[:], AF.Sqrt, bias=eps_row[:])
    rstd_r = rows.tile([1, W], F32, tag="rowC")
    # approx reciprocal (~5x faster): this single-partition row op sits on
    # the serial LN critical path; 51-ULP accuracy is ample for rstd
    nc.vector.reciprocal_approx_fast(out=rstd_r[:], in_=std_r[:])

    # partition-broadcast the two rows on PE: out[128, n] = ones[1,128].T @
    # row[1, n] (rank-1), sliced to <=512-col PSUM banks
    for g0 in range(0, W, 512):
        g1 = min(g0 + 512, W)
        nc.tensor.matmul(mean_b[:, g0:g1], ones_f32[:],
                         mean_r[0:1, g0:g1], start=True, stop=True,
                         skip_group_check=True)
        nc.tensor.matmul(rstd_b[:, g0:g1], ones_f32[:],
                         rstd_r[0:1, g0:g1], start=True, stop=True,
                         skip_group_check=True)

    for c in range(NE):
        t0 = tmp.tile([128, W], F32, tag="t0")
        nc.vector.tensor_sub(t0[:], src[:, c, :], mean_b[:])
        nc.vector.tensor_mul(t0[:], t0[:], rstd_b[:])
        # gain/bias pass on ACT (idle here) instead of DVE: Identity
        # activation computes in*scale + bias with per-partition APs
        nc.scalar.activation(dst[:, c, :], t0[:], AF.Identity,
                             bias=b_ap[:, c:c + 1],
                             scale=g_ap[:, c:c + 1])


def _program(tc, ctx, outT, ins):
    nc = tc.nc
    (xkv, wqs, wks, wvs, wos, wfcs, wprjs, mask_d, gb, bfc_d, bprj_d) = ins

    # long-lived tiles first (pool releases must be LIFO); x loads issue
    # before every other DMA so LN1 starts as early as possible
    h2_pool = ctx.enter_context(tc.tile_pool(name="h2T", bufs=1))
    h2T = h2_pool.tile([128, NE, T], BF)
    g2_pool = ctx.enter_context(tc.tile_pool(name="g2T", bufs=1))
    g2T = g2_pool.tile([128, NE, T], BF)

    const = ctx.enter_context(tc.tile_pool(name="const", bufs=1))
    # first 4 FC weight tiles, prefetched during attention so the MLP
    # phase doesn't stall on the wfcs stream
    wpre_pool = ctx.enter_context(tc.tile_pool(name="wpre", bufs=1))
    wpre = wpre_pool.tile([128, 4, NE, 128], BF)

    hT_pool = tc.alloc_tile_pool(name="hT", bufs=1)
    hT = hT_pool.tile([128, NE, SKV], BF)
    for c in range(NE):
        nc.sync.dma_start(hT[:, c, :], xkv[c])

    ones = const.tile([128, 1], BF)
    nc.vector.memset(ones[:], 1.0)
    ones_f32 = const.tile([1, 128], F32)
    nc.vector.memset(ones_f32[:], 1.0)
    eps_row = const.tile([1, 1], F32)
    nc.vector.memset(eps_row[:], 1e-5)
    # gb cols: 0:8 ln1_g, 8:16 ln1_b, 16:24 ln2_g, 24:32 ln2_b,
    #          32:40 bq, 40:48 bk, 48:56 (unused), 56:64 bo + wo@bv
    gb_sb = const.tile([128, 64], F32)
    nc.sync.dma_start(gb_sb[:], gb[:])
    bfc_sb = const.tile([128, 32], F32)
    nc.sync.dma_start(bfc_sb[:], bfc_d[:])
    bprj_sb = const.tile([128, 8], F32)
    nc.sync.dma_start(bprj_sb[:], bprj_d[:])

    mask_pool = tc.alloc_tile_pool(name="maskp", bufs=1)
    mask_sb = mask_pool.tile([128, N_MASK, MASK_W], BF)
    for j in range(N_MASK):
        nc.sync.dma_start(mask_sb[:, j, :], mask_d[j])

    # ---- Phase 1: LN1 in place: x (loaded into hT) -> hT ---------------
    with ExitStack() as p1:
        _tln(tc, p1, hT, hT, gb_sb[:, 0:8], gb_sb[:, 8:16], SKV, KVGS,
             ones, eps_row, "1", ones_f32=ones_f32)

    # ---- Phase 2: QKV projections --------------------------------------
    kqv = tc.alloc_tile_pool(name="kqv", bufs=1)
    kT = kqv.tile([128, NE, SKV], BF)
    qT = kqv.tile([128, NE, T], BF)
    # V' per head: 64 v columns + 64 ones columns; the AV matmul then
    # leaves the softmax denominator replicated on psum partitions 64:128
    v_sb = kqv.tile([128, NCH, H * 128], BF)
    v4 = v_sb.rearrange("p t (h d) -> p t h d", d=128)
    nc.vector.memset(v4[:, :, :, 64:128], 1.0)

    ws2_pool = tc.alloc_tile_pool(name="ws2", bufs=3)
    attn_pool = tc.alloc_tile_pool(name="attnT", bufs=1)
    attnT = attn_pool.tile([128, NE, T], BF)
    wv_pool = tc.alloc_tile_pool(name="wvall", bufs=1)
    wv_all = wv_pool.tile([128, 16, 512], BF)

    with ExitStack() as p2:
        pj = p2.enter_context(tc.tile_pool(name="pj2", bufs=4, space="PSUM"))
        for (wdram, dst, gsl, bcol) in ((wks, kT, KVGS, 40), (wqs, qT, QGS, 32)):
            for eo in range(NE):
                wt = ws2_pool.tile([128, NE, 128], BF, tag="w")
                nc.sync.dma_start(wt[:], wdram[eo])
                for gs in gsl:
                    n = gs.stop - gs.start
                    ps = pj.tile([128, 512], F32, tag="ps")
                    for c in range(NE):
                        nc.tensor.matmul(ps[:, :n], wt[:, c, :], hT[:, c, gs],
                                         start=(c == 0), stop=(c == NE - 1),
                                         skip_group_check=True)
                    nc.vector.tensor_scalar_add(
                        out=dst[:, eo, gs], in0=ps[:, :n],
                        scalar1=gb_sb[:, bcol + eo:bcol + eo + 1])
        # wv loads emitted after the k/q weight loads so they don't block
        # them on the DMA queue (V matmuls run last; the 4MB load overlaps
        # the K/Q matmuls)
        for g2 in range(2):
            for c in range(NE):
                nc.sync.dma_start(wv_all[:, g2 * NE + c, :], wvs[g2, c])
        # V token-major; e_out groups of 512 = 8 heads each
        for tt in range(NCH):
            for g2 in range(2):
                ps = pj.tile([128, 512], F32, tag="ps")
                for c in range(NE):
                    nc.tensor.matmul(ps[:], hT[:, c, tt * 128:(tt + 1) * 128],
                                     wv_all[:, g2 * NE + c, :],
                                     start=(c == 0), stop=(c == NE - 1),
                                     skip_group_check=True)
                nc.vector.tensor_copy(
                    out=v4[:, tt, g2 * 8:(g2 + 1) * 8, 0:64],
                    in_=ps[:].rearrange("p (h d) -> p h d", d=64))

    # ---- Phase 3: attention ---------------------------------------------
    wv_pool.release()

    # FC weight prefetch: the DMA queue is idle during attention
    for j in range(4):
        nc.sync.dma_start(wpre[:, j], wfcs[j])

    with ExitStack() as p3:
        pss = p3.enter_context(tc.tile_pool(name="psS", bufs=2, space="PSUM"))
        pso = p3.enter_context(tc.tile_pool(name="psO", bufs=4, space="PSUM"))
        ptp = p3.enter_context(tc.tile_pool(name="pT", bufs=4))
        rcp = p3.enter_context(tc.tile_pool(name="rcp", bufs=3))

        # h outer / qg inner: interleaving the sparse-qg0 and dense-qg1
        # PE work keeps the tensor engine busy enough that the HAM clock
        # gate stays at 8/8 through the whole attention phase
        for h in range(H):
            pt, po = h // 2, (h % 2) * 64
            for qg in range(2):
                qs = QGS[qg]
                pairs = PAIRS_QG[qg]
                npair = len(pairs)
                ps_o = pso.tile([128, 512], F32, tag="o")
                for pi, pair in enumerate(pairs):
                    ps_s = pss.tile([128, MASK_W], F32, tag="s")
                    for k, c in enumerate(pair):
                        nc.tensor.matmul(
                            ps_s[:, k * 512:k * 512 + 512],
                            kT[po:po + 64, pt, c * 128:(c + 1) * 128],
                            qT[po:po + 64, pt, qs],
                            start=True, stop=True, skip_group_check=True)
                    w = len(pair) * 512
                    pT = ptp.tile([128, MASK_W], BF, tag="pT")
                    nc.scalar.activation(pT[:, :w], ps_s[:, :w], AF.Exp)
                    nc.vector.tensor_mul(pT[:, :w], pT[:, :w],
                                         mask_sb[:, MASK_J0[qg] + pi, :w])
                    for k, c in enumerate(pair):
                        nc.tensor.matmul(
                            ps_o[:, :],
                            v_sb[:, c, h * 128:h * 128 + 128],
                            pT[:, k * 512:k * 512 + 512],
                            start=(pi == 0 and k == 0),
                            stop=(pi == npair - 1 and k == len(pair) - 1),
                            skip_group_check=True)
                # denominator sits replicated on psum partitions 64:128
                # (ones half of V'): copy to SBUF, approx-reciprocal in
                # place (custom-DVE op needs SBUF input), multiply.
                # bv is host-folded into bo, so no bias add here.
                denb = rcp.tile([64, 512], F32, tag="denb")
                nc.vector.tensor_copy(denb[:], ps_o[64:128, :])
                nc.vector.reciprocal_approx_fast(out=denb[:], in_=denb[:])
                nc.vector.tensor_mul(attnT[po:po + 64, pt, qs],
                                     ps_o[0:64, :], denb[:])

    # ---- Phase 4: Wo + residual -> h2T; LN2 stats fused into the Wo
    # loop (each h2T chunk feeds the LN2 partition-reduce immediately,
    # interleaving stats matmuls with Wo matmuls on the PE)
    with ExitStack() as p45:
        bc2 = p45.enter_context(tc.tile_pool(name="bc2", bufs=1,
                                             space="PSUM"))
        mean_b2 = bc2.tile([128, T], F32, tag="mean_b")
        rstd_b2 = bc2.tile([128, T], F32, tag="rstd_b")
        with ExitStack() as p4:
            pj = p4.enter_context(tc.tile_pool(name="pj4", bufs=4,
                                               space="PSUM"))
            sqp = p4.enter_context(tc.tile_pool(name="sq4", bufs=2))
            for eo in range(NE):
                wt = ws2_pool.tile([128, NE, 128], BF, tag="w")
                nc.sync.dma_start(wt[:], wos[eo])
                for qg in range(2):
                    qs = QGS[qg]
                    ps = pj.tile([128, 512], F32, tag="ps")
                    for c in range(NE):
                        nc.tensor.matmul(ps[:], wt[:, c, :], attnT[:, c, qs],
                                         start=(c == 0), stop=(c == NE - 1),
                                         skip_group_check=True)
                    nc.vector.scalar_tensor_tensor(
                        h2T[:, eo, qs], ps[:], gb_sb[:, 56 + eo:56 + eo + 1],
                        hT[:, eo, qs], op0=OP.add, op1=OP.add)
                sq = sqp.tile([128, T], BF, tag="sq")
                nc.scalar.activation(sq[:], h2T[:, eo, :], AF.Square)
                for gs in QGS:
                    nc.tensor.matmul(mean_b2[0:1, gs], ones[:],
                                     h2T[:, eo, gs], start=(eo == 0),
                                     stop=(eo == NE - 1),
                                     skip_group_check=True)
                    nc.tensor.matmul(rstd_b2[0:1, gs], ones[:], sq[:, gs],
                                     start=(eo == 0), stop=(eo == NE - 1),
                                     skip_group_check=True)

        attn_pool.release()
        ws2_pool.release()
        kqv.release()
        mask_pool.release()
        hT_pool.release()

        with ExitStack() as p5:
            _tln_finish(tc, p5, h2T, g2T, gb_sb[:, 16:24], gb_sb[:, 24:32],
                        T, ones_f32, mean_b2, rstd_b2, eps_row, "2")

    # ---- Phase 5: MLP + residual -> outT -------------------------------
    with ExitStack() as p6:
        ws = p6.enter_context(tc.tile_pool(name="ws6", bufs=3))
        wp = p6.enter_context(tc.tile_pool(name="wp6", bufs=3))
        pj = p6.enter_context(tc.tile_pool(name="pj6", bufs=4, space="PSUM"))
        up = p6.enter_context(tc.tile_pool(name="uT", bufs=1))
        op_ = p6.enter_context(tc.tile_pool(name="outp", bufs=3))
        uT = up.tile([128, NHT, T], BF, tag="uT")
        for ht in range(NHT):
            if ht < 4:
                wt = wpre[:, ht]
            else:
                wt = ws.tile([128, NE, 128], BF, tag="w")
                nc.sync.dma_start(wt[:], wfcs[ht])
            for qg in range(2):
                qs = QGS[qg]
                ps = pj.tile([128, 512], F32, tag="ps")
                for c in range(NE):
                    nc.tensor.matmul(ps[:], wt[:, c, :], g2T[:, c, qs],
                                     start=(c == 0), stop=(c == NE - 1),
                                     skip_group_check=True)
                if not SIM_SAFE_GELU:
                    nc.scalar.activation(uT[:, ht, qs], ps[:],
                                         AF.Gelu_apprx_tanh,
                                         bias=bfc_sb[:, ht:ht + 1])
                else:
                    # new_gelu(u) = u * sigmoid(2c*(u + 0.044715 u^3))
                    u_sb = ws.tile([128, 512], BF, tag="gelu_u")
                    nc.vector.tensor_scalar_add(
                        out=u_sb[:], in0=ps[:],
                        scalar1=bfc_sb[:, ht:ht + 1])
                    t = ws.tile([128, 512], BF, tag="gelu_t")
                    nc.scalar.activation(t[:], u_sb[:], AF.Square)
                    z = ws.tile([128, 512], BF, tag="gelu_z")
                    nc.vector.scalar_tensor_tensor(
                        z[:], t[:], 0.044715, u_sb[:],
                        op0=OP.mult, op1=OP.mult)
                    nc.vector.tensor_add(z[:], z[:], u_sb[:])
                    g = ws.tile([128, 512], BF, tag="gelu_g")
                    nc.scalar.activation(
                        g[:], z[:], AF.Sigmoid,
                        scale=float(2.0 * np.sqrt(2.0 / np.pi)))
                    nc.vector.tensor_mul(uT[:, ht, qs], u_sb[:], g[:])
        for eo in range(NE):
            wt = wp.tile([128, NHT, 128], BF, tag="w")
            nc.sync.dma_start(wt[:], wprjs[eo])
            for qg in range(2):
                qs = QGS[qg]
                ps = pj.tile([128, 512], F32, tag="ps")
                for c in range(NHT):
                    nc.tensor.matmul(ps[:], wt[:, c, :], uT[:, c, qs],
                                     start=(c == 0), stop=(c == NHT - 1),
                                     skip_group_check=True)
                ot = op_.tile([128, 512], F32, tag="ot")
                nc.vector.scalar_tensor_tensor(
                    ot[:], ps[:], bprj_sb[:, eo:eo + 1], h2T[:, eo, qs],
                    op0=OP.add, op1=OP.add)
                nc.sync.dma_start(outT[eo][:, qs], ot[:])


def _build():
    if "nc" in _CACHE:
        return _CACHE["nc"]
    nc = bacc.Bacc("TRN2", target_bir_lowering=False, debug=False,
                   num_devices=N_CORES)

    def din(name, shape, dt=BF):
        return nc.dram_tensor(name, shape, dt, kind="ExternalInput").ap()

    ins = [
        din("xkv", [NE, 128, SKV]),
        din("wqs", [NE, 128, NE, 128]),
        din("wks", [NE, 128, NE, 128]),
        din("wvs", [2, NE, 128, 512]),
        din("wos", [NE, 128, NE, 128]),
        din("wfcs", [NHT, 128, NE, 128]),
        din("wprjs", [NE, 128, NHT, 128]),
        din("mask", [N_MASK, 128, MASK_W]),
        din("gb", [128, 64], F32),
        din("bfc", [128, 32], F32),
        din("bprj", [128, 8], F32),
    ]
    outT = nc.dram_tensor("outT", [NE, 128, T], F32, kind="ExternalOutput").ap()

    with tile.TileContext(nc) as tc:
        with ExitStack() as ctx:
            _program(tc, ctx, outT, ins)
    nc.compile()
    _CACHE["nc"] = nc
    return nc


# ------------------------------------------------------------------ host code

def _prep_shared(args):
    (ln1_g, ln1_b, ln2_g, ln2_b, wq, bq, wk, bk, wv, bv, wo, bo,
     w_fc, b_fc, w_proj, b_proj) = [np.asarray(a, np.float32) for a in args]
    bf = ml_dtypes.bfloat16

    def eo_layout(wT, nk, nm):  # wT [nk*128, nm*128] -> [nm, 128, nk, 128]
        return np.ascontiguousarray(
            wT.reshape(nk, 128, nm, 128).transpose(2, 1, 0, 3).astype(bf))

    def cols(v, n):  # [n*128] -> [128, n]
        return np.ascontiguousarray(
            np.asarray(v, np.float32).reshape(n, 128).T)

    scale = np.float32(D ** -0.5)
    d = {}
    d["wqs"] = eo_layout((wq * scale).T, NE, NE)
    d["wks"] = eo_layout(wk.T, NE, NE)
    d["wvs"] = np.ascontiguousarray(
        wv.T.reshape(NE, 128, 2, 512).transpose(2, 0, 1, 3).astype(bf))
    d["wos"] = eo_layout(wo.T, NE, NE)
    d["wfcs"] = eo_layout(w_fc.T, NE, NHT)
    d["wprjs"] = eo_layout(w_proj.T, NHT, NE)

    gb = np.zeros((128, 64), np.float32)
    gb[:, 0:8] = cols(ln1_g, 8)
    gb[:, 8:16] = cols(ln1_b, 8)
    gb[:, 16:24] = cols(ln2_g, 8)
    gb[:, 24:32] = cols(ln2_b, 8)
    gb[:, 32:40] = cols(bq * scale, 8)
    gb[:, 40:48] = cols(bk, 8)
    # bv folded into bo: attn_out = (softmax(..)V + bv) Wo^T + bo
    #                             = softmax(..)V Wo^T + (bo + Wo bv)
    gb[:, 56:64] = cols(bo + wo @ bv, 8)
    d["gb"] = gb
    d["bfc"] = cols(b_fc, 32)
    d["bprj"] = cols(b_proj, 8)
    return d


def _core_inputs(x, mask, shared, core):
    bf = ml_dtypes.bfloat16
    b, half = divmod(core, 2)
    qtok = np.arange(half * T, (half + 1) * T)
    need = np.where(mask[qtok].any(axis=0))[0]
    extra = np.setdiff1d(need, qtok)
    nreal = T + len(extra)
    assert nreal <= SKV, (core, nreal)
    kv = np.concatenate([qtok, extra,
                         np.zeros(SKV - nreal, np.int64)])

    xkvT = x[b].T[:, kv].astype(bf)  # [E, SKV]
    m = np.zeros((T, SKV), np.float32)
    m[:, :nreal] = mask[np.ix_(qtok, kv[:nreal])]
    mT = m.T  # [SKV, T]
    mtiles = np.zeros((N_MASK, 128, MASK_W), np.float32)
    for qg in range(2):
        active = {c for pair in PAIRS_QG[qg] for c in pair}
        skipped = [c for c in range(NCH) if c not in active]
        for c in skipped:
            assert not mT[c * 128:(c + 1) * 128,
                          qg * 512:(qg + 1) * 512].any(), (core, qg, c)
        for pi, pair in enumerate(PAIRS_QG[qg]):
            for k, c in enumerate(pair):
                mtiles[MASK_J0[qg] + pi, :, k * 512:(k + 1) * 512] = \
                    mT[c * 128:(c + 1) * 128, qg * 512:(qg + 1) * 512]

    im = dict(shared)
    im["xkv"] = np.ascontiguousarray(xkvT.reshape(NE, 128, SKV))
    im["mask"] = np.ascontiguousarray(mtiles.astype(bf))
    return im, qtok


def kernel(x, ln1_g, ln1_b, ln2_g, ln2_b, wq, bq, wk, bk, wv, bv, wo, bo,
           w_fc, b_fc, w_proj, b_proj, mask):
    x = np.asarray(x, np.float32)
    mask = np.asarray(mask, bool)

    nc = _build()
    shared = _prep_shared((ln1_g, ln1_b, ln2_g, ln2_b, wq, bq, wk, bk, wv,
                           bv, wo, bo, w_fc, b_fc, w_proj, b_proj))

    in_maps, qtoks = [], []
    for core in range(N_CORES):
        im, qtok = _core_inputs(x, mask, shared, core)
        in_maps.append(im)
        qtoks.append(qtok)

    br = bass_utils.run_bass_kernel_spmd(nc, in_maps,
                                         core_ids=list(range(N_CORES)))
    out = np.empty((B, S, E), np.float32)
    for core in range(N_CORES):
        b, half = divmod(core, 2)
        oT = br.results[core]["outT"]  # [NE, 128, T]
        out[b, qtoks[core], :] = oT.reshape(E, T).T
    return out



# revision 30
# speedup vs baseline: 1.0371x; 1.0371x over previous
"""Sparse-attention transformer block on 8 Trainium2 NeuronCores (Bass/Tile).

Sharding: 8 cores = 4 batches x 2 query-halves (SPMD, one program).
Each core processes T=1024 query tokens of one batch. Key/value tokens are
host-gathered per core: the core's own 1024 tokens plus the (static-mask)
summary tokens its queries attend outside that range, padded to SKV=1408.
All activations are feature-major ("transposed", [feature, token]) so every
matmul contracts along partitions with zero on-device transposes:

  xT -> LN1 (partition reduce via ones-matmul) -> hT
  kT = Wk hT, qT = Wq hT (feature-major); V = hT^T Wv^T (token-major)
  scoresT[s,q] = kT^T qT per head; p = exp(s) * mask01 (scores are small:
  no max subtraction needed); oT[d,q] = V'^T p with a ones column in V'
  giving the softmax denominator for free; normalize, Wo, residual, LN2,
  MLP (gelu-tanh), residual -> outT.

Matmuls run in bf16 (tolerance 2e-2 >> bf16 error).
"""

import numpy as np
import ml_dtypes
from contextlib import ExitStack

import concourse.bass as bass
import concourse.bacc as bacc
import concourse.tile as tile
from concourse import mybir
from concourse import bass_utils

B, S, E, H, D = 4, 2048, 1024, 16, 64
HID = 4 * E
T = 1024            # query tokens per core
SKV = 1408          # gathered kv tokens per core (11 chunks of 128)
NCH = SKV // 128    # 11 s-chunks
NE = E // 128       # 8 feature chunks
NHT = HID // 128    # 32 hidden chunks
N_CORES = 8
# Active s-chunk pairs per query group. For queries q < 512 (qg0), keys are
# causally <= 511 (chunks 0-3) plus gathered summary chunks 8-10; chunks 4-7
# are fully masked for every core (host asserts this).
PAIRS_QG = [
    [(0, 1), (2, 3), (8, 9), (10,)],
    [(0, 1), (2, 3), (4, 5), (6, 7), (8, 9), (10,)],
]
MASK_J0 = [0, len(PAIRS_QG[0])]           # mask tile base index per qg
N_MASK = len(PAIRS_QG[0]) + len(PAIRS_QG[1])
MASK_W = 1024
QGS = [slice(0, 512), slice(512, 1024)]
KVGS = [slice(0, 512), slice(512, 1024), slice(1024, 1408)]
BF = mybir.dt.bfloat16
F32 = mybir.dt.float32
AF = mybir.ActivationFunctionType
OP = mybir.AluOpType

_CACHE = {}
import os
SIM_SAFE_GELU = os.environ.get("BASS_SIM_SAFE_GELU") == "1"


# ---------------------------------------------------------------- device code

def _tln(tc, ctx, src, dst, g_ap, b_ap, W, groups, ones, eps_row, tag,
         ones_f32=None):
    """Transposed layernorm: src [128, NE, W] bf16 -> dst [128, NE, W] bf16.

    Stats are partition reductions via ones-matmuls; mean/rstd rows get
    partition-broadcast via rank-1 PE matmuls (ones_f32 outer product) into
    PSUM -- no DRAM bounce. g_ap/b_ap: [128, NE]."""
    nc = tc.nc
    tmp = ctx.enter_context(tc.tile_pool(name=f"tmp{tag}", bufs=2))
    rows = ctx.enter_context(tc.tile_pool(name=f"rows{tag}", bufs=1))
    bc = ctx.enter_context(tc.tile_pool(name=f"bc{tag}", bufs=1, space="PSUM"))

    # stats rows live in partition 0 of the (later) broadcast tiles: the
    # row chain consumes them before the rank-1 matmuls overwrite the banks
    mean_b = bc.tile([128, W], F32, tag="mean_b")
    rstd_b = bc.tile([128, W], F32, tag="rstd_b")
    ps_mean = mean_b[0:1, :]
    ps_sq = rstd_b[0:1, :]
    for gs in groups:
        for c in range(NE):
            nc.tensor.matmul(ps_mean[:, gs], ones[:], src[:, c, gs],
                             start=(c == 0), stop=(c == NE - 1),
                             skip_group_check=True)
    for c in range(NE):
        sq = tmp.tile([128, W], BF, tag="sq")
        nc.scalar.activation(sq[:], src[:, c, :], AF.Square)
        for gs in groups:
            nc.tensor.matmul(ps_sq[:, gs], ones[:], sq[:, gs],
                             start=(c == 0), stop=(c == NE - 1),
                             skip_group_check=True)

    _tln_finish(tc, ctx, src, dst, g_ap, b_ap, groups, ones_f32, mean_b,
                rstd_b, eps_row, tag)


def _tln_finish(tc, ctx, src, dst, g_ap, b_ap, groups, ones_f32, mean_b,
                rstd_b, eps_row, tag):
    """LN tail: raw sums sit in partition 0 of mean_b/rstd_b (PSUM).
    Per token-group: compute mean/rstd rows, rank-1 partition-broadcast
    them back into the same banks, normalize src -> dst. Group-at-a-time
    so downstream consumers of dst[:, :, g] unblock before later groups."""
    nc = tc.nc
    tmp = ctx.enter_context(tc.tile_pool(name=f"tmpf{tag}", bufs=4))
    rows = ctx.enter_context(tc.tile_pool(name=f"rows{tag}", bufs=2))
    ps_mean = mean_b[0:1, :]
    ps_sq = rstd_b[0:1, :]

    for gs in groups:
        n = gs.stop - gs.start
        mean_r = rows.tile([1, 512], F32, tag="rowA")
        nc.scalar.mul(mean_r[:, :n], ps_mean[:, gs], 1.0 / E)
        msq_r = rows.tile([1, 512], F32, tag="rowB")
        nc.vector.tensor_mul(msq_r[:, :n], mean_r[:, :n], mean_r[:, :n])
        var_r = rows.tile([1, 512], F32, tag="rowC")
        nc.vector.scalar_tensor_tensor(var_r[:, :n], ps_sq[:, gs], 1.0 / E,
                                       msq_r[:, :n],
                                       op0=OP.mult, op1=OP.subtract)
        std_r = rows.tile([1, 512], F32, tag="rowD")
        nc.scalar.activation(std_r[:, :n], var_r[:, :n], AF.Sqrt,
                             bias=eps_row[:])
        rstd_r = rows.tile([1, 512], F32, tag="rowE")
        # approx reciprocal (~5x faster): this single-partition row op sits
        # on the serial LN critical path; 51 ULP is ample for rstd
        nc.vector.reciprocal_approx_fast(out=rstd_r[:, :n], in_=std_r[:, :n])

        # partition-broadcast the two rows on PE: out[128, n] =
        # ones[1,128].T @ row[1, n] (rank-1) into the group's PSUM bank
        nc.tensor.matmul(mean_b[:, gs], ones_f32[:], mean_r[0:1, :n],
                         start=True, stop=True, skip_group_check=True)
        nc.tensor.matmul(rstd_b[:, gs], ones_f32[:], rstd_r[0:1, :n],
                         start=True, stop=True, skip_group_check=True)

        for c in range(NE):
            t0 = tmp.tile([128, 512], F32, tag="t0")
            nc.vector.tensor_sub(t0[:, :n], src[:, c, gs], mean_b[:, gs])
            nc.vector.tensor_mul(t0[:, :n], t0[:, :n], rstd_b[:, gs])
            # gain/bias pass on ACT (idle here) instead of DVE: Identity
            # activation computes in*scale + bias with per-partition APs
            nc.scalar.activation(dst[:, c, gs], t0[:, :n], AF.Identity,
                                 bias=b_ap[:, c:c + 1],
                                 scale=g_ap[:, c:c + 1])


def _program(tc, ctx, outT, ins):
    nc = tc.nc
    (xkv, wqs, wks, wvs, wos, wfcs, wprjs, mask_d, gb, bfc_d, bprj_d) = ins

    # long-lived tiles first (pool releases must be LIFO); x loads issue
    # before every other DMA so LN1 starts as early as possible
    h2_pool = ctx.enter_context(tc.tile_pool(name="h2T", bufs=1))
    h2T = h2_pool.tile([128, NE, T], BF)
    g2_pool = ctx.enter_context(tc.tile_pool(name="g2T", bufs=1))
    g2T = g2_pool.tile([128, NE, T], BF)

    const = ctx.enter_context(tc.tile_pool(name="const", bufs=1))

    hT_pool = tc.alloc_tile_pool(name="hT", bufs=1)
    hT = hT_pool.tile([128, NE, SKV], BF)
    for c in range(NE):
        nc.sync.dma_start(hT[:, c, :], xkv[c])

    ones = const.tile([128, 1], BF)
    nc.vector.memset(ones[:], 1.0)
    ones_f32 = const.tile([1, 128], F32)
    nc.vector.memset(ones_f32[:], 1.0)
    eps_row = const.tile([1, 1], F32)
    nc.vector.memset(eps_row[:], 1e-5)
    # preload the exp/gelu activation tables now (ACT is idle during the
    # x DMA): a table load at the QKV->attention transition stalls exp,
    # idles the PE >3.4us, and can latch the HAM clock gate at 4/8 for
    # the entire attention phase
    warm = const.tile([1, 2], F32)
    nc.scalar.activation(warm[:, 0:1], eps_row[:], AF.Exp)
    nc.scalar.activation(warm[:, 1:2], eps_row[:], AF.Gelu_apprx_tanh)
    # gb cols: 0:8 ln1_g, 8:16 ln1_b, 16:24 ln2_g, 24:32 ln2_b,
    #          32:40 bq, 40:48 bk, 48:56 (unused), 56:64 bo + wo@bv
    gb_sb = const.tile([128, 64], F32)
    nc.sync.dma_start(gb_sb[:], gb[:])
    bfc_sb = const.tile([128, 32], F32)
    nc.sync.dma_start(bfc_sb[:], bfc_d[:])
    bprj_sb = const.tile([128, 8], F32)
    nc.sync.dma_start(bprj_sb[:], bprj_d[:])

    mask_pool = tc.alloc_tile_pool(name="maskp", bufs=1)
    mask_sb = mask_pool.tile([128, N_MASK, MASK_W], BF)

    # ---- Phase 1: LN1 in place: x (loaded into hT) -> hT ---------------
    with ExitStack() as p1:
        _tln(tc, p1, hT, hT, gb_sb[:, 0:8], gb_sb[:, 8:16], SKV, KVGS,
             ones, eps_row, "1", ones_f32=ones_f32)

    # ---- Phase 2: QKV projections --------------------------------------
    kqv = tc.alloc_tile_pool(name="kqv", bufs=1)
    kT = kqv.tile([128, NE, SKV], BF)
    qT = kqv.tile([128, NE, T], BF)
    # V' per head: 64 v columns + 64 ones columns; the AV matmul then
    # leaves the softmax denominator replicated on psum partitions 64:128
    v_sb = kqv.tile([128, NCH, H * 128], BF)
    v4 = v_sb.rearrange("p t (h d) -> p t h d", d=128)
    nc.vector.memset(v4[:, :, :, 64:128], 1.0)

    ws2_pool = tc.alloc_tile_pool(name="ws2", bufs=8)
    attn_pool = tc.alloc_tile_pool(name="attnT", bufs=1)
    attnT = attn_pool.tile([128, NE, T], BF)
    wv_pool = tc.alloc_tile_pool(name="wvall", bufs=1)
    wv_all = wv_pool.tile([128, 16, 512], BF)

    with ExitStack() as p2:
        pj = p2.enter_context(tc.tile_pool(name="pj2", bufs=4, space="PSUM"))

        def proj(wdram, dst, gsl, bcol, eo):
            wt = ws2_pool.tile([128, NE, 128], BF, tag="w")
            nc.sync.dma_start(wt[:], wdram[eo])
            for gs in gsl:
                n = gs.stop - gs.start
                ps = pj.tile([128, 512], F32, tag="ps")
                for c in range(NE):
                    nc.tensor.matmul(ps[:, :n], wt[:, c, :], hT[:, c, gs],
                                     start=(c == 0), stop=(c == NE - 1),
                                     skip_group_check=True)
                nc.vector.tensor_scalar_add(
                    out=dst[:, eo, gs], in0=ps[:, :n],
                    scalar1=gb_sb[:, bcol + eo:bcol + eo + 1])

        for eo in range(NE):
            proj(wks, kT, KVGS, 40, eo)
        # wv load emitted after the k-weight loads so it doesn't block them
        # on the DMA queue (V matmuls run last; the 4MB load overlaps K/Q)
        for g2 in range(2):
            for c in range(NE):
                nc.sync.dma_start(wv_all[:, g2 * NE + c, :], wvs[g2, c])
        for eo in range(NE):
            proj(wqs, qT, QGS, 32, eo)
        # masks: needed only at attention start
        for j in range(N_MASK):
            nc.sync.dma_start(mask_sb[:, j, :], mask_d[j])
        # V token-major; e_out groups of 512 = 8 heads each
        for tt in range(NCH):
            for g2 in range(2):
                ps = pj.tile([128, 512], F32, tag="ps")
                for c in range(NE):
                    nc.tensor.matmul(ps[:], hT[:, c, tt * 128:(tt + 1) * 128],
                                     wv_all[:, g2 * NE + c, :],
                                     start=(c == 0), stop=(c == NE - 1),
                                     skip_group_check=True)
                nc.vector.tensor_copy(
                    out=v4[:, tt, g2 * 8:(g2 + 1) * 8, 0:64],
                    in_=ps[:].rearrange("p (h d) -> p h d", d=64))

    # ---- Phase 3: attention ---------------------------------------------
    wv_pool.release()

    with ExitStack() as p3:
        pss = p3.enter_context(tc.tile_pool(name="psS", bufs=2, space="PSUM"))
        pso = p3.enter_context(tc.tile_pool(name="psO", bufs=4, space="PSUM"))
        ptp = p3.enter_context(tc.tile_pool(name="pT", bufs=4))
        rcp = p3.enter_context(tc.tile_pool(name="rcp", bufs=3))

        # h outer / qg inner: interleaving the sparse-qg0 and dense-qg1
        # PE work keeps the tensor engine busy enough that the HAM clock
        # gate stays at 8/8 through the whole attention phase
        for h in range(H):
            pt, po = h // 2, (h % 2) * 64
            for qg in range(2):
                qs = QGS[qg]
                pairs = PAIRS_QG[qg]
                npair = len(pairs)
                ps_o = pso.tile([128, 512], F32, tag="o")
                for pi, pair in enumerate(pairs):
                    ps_s = pss.tile([128, MASK_W], F32, tag="s")
                    for k, c in enumerate(pair):
                        nc.tensor.matmul(
                            ps_s[:, k * 512:k * 512 + 512],
                            kT[po:po + 64, pt, c * 128:(c + 1) * 128],
                            qT[po:po + 64, pt, qs],
                            start=True, stop=True, skip_group_check=True)
                    w = len(pair) * 512
                    pT = ptp.tile([128, MASK_W], BF, tag="pT")
                    nc.scalar.activation(pT[:, :w], ps_s[:, :w], AF.Exp)
                    nc.vector.tensor_mul(pT[:, :w], pT[:, :w],
                                         mask_sb[:, MASK_J0[qg] + pi, :w])
                    for k, c in enumerate(pair):
                        nc.tensor.matmul(
                            ps_o[:, :],
                            v_sb[:, c, h * 128:h * 128 + 128],
                            pT[:, k * 512:k * 512 + 512],
                            start=(pi == 0 and k == 0),
                            stop=(pi == npair - 1 and k == len(pair) - 1),
                            skip_group_check=True)
                # denominator sits replicated on psum partitions 64:128
                # (ones half of V'): copy to SBUF, approx-reciprocal in
                # place (custom-DVE op needs SBUF input), multiply.
                # bv is host-folded into bo, so no bias add here.
                denb = rcp.tile([64, 512], F32, tag="denb")
                nc.vector.tensor_copy(denb[:], ps_o[64:128, :])
                nc.vector.reciprocal_approx_fast(out=denb[:], in_=denb[:])
                nc.vector.tensor_mul(attnT[po:po + 64, pt, qs],
                                     ps_o[0:64, :], denb[:])

    # ---- Phase 4: Wo + residual -> h2T; LN2 stats fused into the Wo
    # loop (each h2T chunk feeds the LN2 partition-reduce immediately,
    # interleaving stats matmuls with Wo matmuls on the PE)
    with ExitStack() as p45:
        bc2 = p45.enter_context(tc.tile_pool(name="bc2", bufs=1,
                                             space="PSUM"))
        mean_b2 = bc2.tile([128, T], F32, tag="mean_b")
        rstd_b2 = bc2.tile([128, T], F32, tag="rstd_b")
        with ExitStack() as p4:
            pj = p4.enter_context(tc.tile_pool(name="pj4", bufs=4,
                                               space="PSUM"))
            sqp = p4.enter_context(tc.tile_pool(name="sq4", bufs=2))
            for eo in range(NE):
                wt = ws2_pool.tile([128, NE, 128], BF, tag="w")
                nc.sync.dma_start(wt[:], wos[eo])
                for qg in range(2):
                    qs = QGS[qg]
                    ps = pj.tile([128, 512], F32, tag="ps")
                    for c in range(NE):
                        nc.tensor.matmul(ps[:], wt[:, c, :], attnT[:, c, qs],
                                         start=(c == 0), stop=(c == NE - 1),
                                         skip_group_check=True)
                    nc.vector.scalar_tensor_tensor(
                        h2T[:, eo, qs], ps[:], gb_sb[:, 56 + eo:56 + eo + 1],
                        hT[:, eo, qs], op0=OP.add, op1=OP.add)
                sq = sqp.tile([128, T], BF, tag="sq")
                nc.scalar.activation(sq[:], h2T[:, eo, :], AF.Square)
                for gs in QGS:
                    nc.tensor.matmul(mean_b2[0:1, gs], ones[:],
                                     h2T[:, eo, gs], start=(eo == 0),
                                     stop=(eo == NE - 1),
                                     skip_group_check=True)
                    nc.tensor.matmul(rstd_b2[0:1, gs], ones[:], sq[:, gs],
                                     start=(eo == 0), stop=(eo == NE - 1),
                                     skip_group_check=True)

        attn_pool.release()
        ws2_pool.release()
        kqv.release()
        mask_pool.release()
        hT_pool.release()

        with ExitStack() as p5:
            _tln_finish(tc, p5, h2T, g2T, gb_sb[:, 16:24], gb_sb[:, 24:32],
                        QGS, ones_f32, mean_b2, rstd_b2, eps_row, "2")

    # ---- Phase 5: MLP + residual -> outT -------------------------------
    with ExitStack() as p6:
        ws = p6.enter_context(tc.tile_pool(name="ws6", bufs=8))
        wp = p6.enter_context(tc.tile_pool(name="wp6", bufs=3))
        pj = p6.enter_context(tc.tile_pool(name="pj6", bufs=4, space="PSUM"))
        up = p6.enter_context(tc.tile_pool(name="uT", bufs=1))
        op_ = p6.enter_context(tc.tile_pool(name="outp", bufs=3))
        uT = up.tile([128, NHT, T], BF, tag="uT")
        for ht in range(NHT):
            wt = ws.tile([128, NE, 128], BF, tag="w")
            nc.sync.dma_start(wt[:], wfcs[ht])
            for qg in range(2):
                qs = QGS[qg]
                ps = pj.tile([128, 512], F32, tag="ps")
                for c in range(NE):
                    nc.tensor.matmul(ps[:], wt[:, c, :], g2T[:, c, qs],
                                     start=(c == 0), stop=(c == NE - 1),
                                     skip_group_check=True)
                if not SIM_SAFE_GELU:
                    nc.scalar.activation(uT[:, ht, qs], ps[:],
                                         AF.Gelu_apprx_tanh,
                                         bias=bfc_sb[:, ht:ht + 1])
                else:
                    # new_gelu(u) = u * sigmoid(2c*(u + 0.044715 u^3))
                    u_sb = ws.tile([128, 512], BF, tag="gelu_u")
                    nc.vector.tensor_scalar_add(
                        out=u_sb[:], in0=ps[:],
                        scalar1=bfc_sb[:, ht:ht + 1])
                    t = ws.tile([128, 512], BF, tag="gelu_t")
                    nc.scalar.activation(t[:], u_sb[:], AF.Square)
                    z = ws.tile([128, 512], BF, tag="gelu_z")
                    nc.vector.scalar_tensor_tensor(
                        z[:], t[:], 0.044715, u_sb[:],
                        op0=OP.mult, op1=OP.mult)
                    nc.vector.tensor_add(z[:], z[:], u_sb[:])
                    g = ws.tile([128, 512], BF, tag="gelu_g")
                    nc.scalar.activation(
                        g[:], z[:], AF.Sigmoid,
                        scale=float(2.0 * np.sqrt(2.0 / np.pi)))
                    nc.vector.tensor_mul(uT[:, ht, qs], u_sb[:], g[:])
        for eo in range(NE):
            wt = wp.tile([128, NHT, 128], BF, tag="w")
            nc.sync.dma_start(wt[:], wprjs[eo])
            for qg in range(2):
                qs = QGS[qg]
                ps = pj.tile([128, 512], F32, tag="ps")
                for c in range(NHT):
                    nc.tensor.matmul(ps[:], wt[:, c, :], uT[:, c, qs],
                                     start=(c == 0), stop=(c == NHT - 1),
                                     skip_group_check=True)
                ot = op_.tile([128, 512], F32, tag="ot")
                nc.vector.scalar_tensor_tensor(
                    ot[:], ps[:], bprj_sb[:, eo:eo + 1], h2T[:, eo, qs],
                    op0=OP.add, op1=OP.add)
                nc.sync.dma_start(outT[eo][:, qs], ot[:])


def _build():
    if "nc" in _CACHE:
        return _CACHE["nc"]
    nc = bacc.Bacc("TRN2", target_bir_lowering=False, debug=False,
                   num_devices=N_CORES)

    def din(name, shape, dt=BF):
        return nc.dram_tensor(name, shape, dt, kind="ExternalInput").ap()

    ins = [
        din("xkv", [NE, 128, SKV]),
        din("wqs", [NE, 128, NE, 128]),
        din("wks", [NE, 128, NE, 128]),
        din("wvs", [2, NE, 128, 512]),
        din("wos", [NE, 128, NE, 128]),
        din("wfcs", [NHT, 128, NE, 128]),
        din("wprjs", [NE, 128, NHT, 128]),
        din("mask", [N_MASK, 128, MASK_W]),
        din("gb", [128, 64], F32),
        din("bfc", [128, 32], F32),
        din("bprj", [128, 8], F32),
    ]
    outT = nc.dram_tensor("outT", [NE, 128, T], F32, kind="ExternalOutput").ap()

    with tile.TileContext(nc) as tc:
        with ExitStack() as ctx:
            _program(tc, ctx, outT, ins)
    nc.compile()
    _CACHE["nc"] = nc
    return nc


# ------------------------------------------------------------------ host code

def _prep_shared(args):
    (ln1_g, ln1_b, ln2_g, ln2_b, wq, bq, wk, bk, wv, bv, wo, bo,
     w_fc, b_fc, w_proj, b_proj) = [np.asarray(a, np.float32) for a in args]
    bf = ml_dtypes.bfloat16

    def eo_layout(wT, nk, nm):  # wT [nk*128, nm*128] -> [nm, 128, nk, 128]
        return np.ascontiguousarray(
            wT.reshape(nk, 128, nm, 128).transpose(2, 1, 0, 3).astype(bf))

    def cols(v, n):  # [n*128] -> [128, n]
        return np.ascontiguousarray(
            np.asarray(v, np.float32).reshape(n, 128).T)

    scale = np.float32(D ** -0.5)
    d = {}
    d["wqs"] = eo_layout((wq * scale).T, NE, NE)
    d["wks"] = eo_layout(wk.T, NE, NE)
    d["wvs"] = np.ascontiguousarray(
        wv.T.reshape(NE, 128, 2, 512).transpose(2, 0, 1, 3).astype(bf))
    d["wos"] = eo_layout(wo.T, NE, NE)
    d["wfcs"] = eo_layout(w_fc.T, NE, NHT)
    d["wprjs"] = eo_layout(w_proj.T, NHT, NE)

    gb = np.zeros((128, 64), np.float32)
    gb[:, 0:8] = cols(ln1_g, 8)
    gb[:, 8:16] = cols(ln1_b, 8)
    gb[:, 16:24] = cols(ln2_g, 8)
    gb[:, 24:32] = cols(ln2_b, 8)
    gb[:, 32:40] = cols(bq * scale, 8)
    gb[:, 40:48] = cols(bk, 8)
    # bv folded into bo: attn_out = (softmax(..)V + bv) Wo^T + bo
    #                             = softmax(..)V Wo^T + (bo + Wo bv)
    gb[:, 56:64] = cols(bo + wo @ bv, 8)
    d["gb"] = gb
    d["bfc"] = cols(b_fc, 32)
    d["bprj"] = cols(b_proj, 8)
    return d


def _core_inputs(x, mask, shared, core):
    bf = ml_dtypes.bfloat16
    b, half = divmod(core, 2)
    qtok = np.arange(half * T, (half + 1) * T)
    need = np.where(mask[qtok].any(axis=0))[0]
    extra = np.setdiff1d(need, qtok)
    nreal = T + len(extra)
    assert nreal <= SKV, (core, nreal)
    kv = np.concatenate([qtok, extra,
                         np.zeros(SKV - nreal, np.int64)])

    xkvT = x[b].T[:, kv].astype(bf)  # [E, SKV]
    m = np.zeros((T, SKV), np.float32)
    m[:, :nreal] = mask[np.ix_(qtok, kv[:nreal])]
    mT = m.T  # [SKV, T]
    mtiles = np.zeros((N_MASK, 128, MASK_W), np.float32)
    for qg in range(2):
        active = {c for pair in PAIRS_QG[qg] for c in pair}
        skipped = [c for c in range(NCH) if c not in active]
        for c in skipped:
            assert not mT[c * 128:(c + 1) * 128,
                          qg * 512:(qg + 1) * 512].any(), (core, qg, c)
        for pi, pair in enumerate(PAIRS_QG[qg]):
            for k, c in enumerate(pair):
                mtiles[MASK_J0[qg] + pi, :, k * 512:(k + 1) * 512] = \
                    mT[c * 128:(c + 1) * 128, qg * 512:(qg + 1) * 512]

    im = dict(shared)
    im["xkv"] = np.ascontiguousarray(xkvT.reshape(NE, 128, SKV))
    im["mask"] = np.ascontiguousarray(mtiles.astype(bf))
    return im, qtok


def kernel(x, ln1_g, ln1_b, ln2_g, ln2_b, wq, bq, wk, bk, wv, bv, wo, bo,
           w_fc, b_fc, w_proj, b_proj, mask):
    x = np.asarray(x, np.float32)
    mask = np.asarray(mask, bool)

    nc = _build()
    shared = _prep_shared((ln1_g, ln1_b, ln2_g, ln2_b, wq, bq, wk, bk, wv,
                           bv, wo, bo, w_fc, b_fc, w_proj, b_proj))

    in_maps, qtoks = [], []
    for core in range(N_CORES):
        im, qtok = _core_inputs(x, mask, shared, core)
        in_maps.append(im)
        qtoks.append(qtok)

    br = bass_utils.run_bass_kernel_spmd(nc, in_maps,
                                         core_ids=list(range(N_CORES)))
    out = np.empty((B, S, E), np.float32)
    for core in range(N_CORES):
        b, half = divmod(core, 2)
        oT = br.results[core]["outT"]  # [NE, 128, T]
        out[b, qtoks[core], :] = oT.reshape(E, T).T
    return out



# revision 33
# speedup vs baseline: 1.1052x; 1.0656x over previous
"""Sparse-attention transformer block on 8 Trainium2 NeuronCores (Bass/Tile).

Sharding: 8 cores = 4 batches x 2 query-halves (SPMD, one program).
Each core processes T=1024 query tokens of one batch. Key/value tokens are
host-gathered per core: the core's own 1024 tokens plus the (static-mask)
summary tokens its queries attend outside that range, padded to SKV=1408.
All activations are feature-major ("transposed", [feature, token]) so every
matmul contracts along partitions with zero on-device transposes:

  xT -> LN1 (partition reduce via ones-matmul) -> hT
  kT = Wk hT, qT = Wq hT (feature-major); V = hT^T Wv^T (token-major)
  scoresT[s,q] = kT^T qT per head; p = exp(s) * mask01 (scores are small:
  no max subtraction needed); oT[d,q] = V'^T p with a ones column in V'
  giving the softmax denominator for free; normalize, Wo, residual, LN2,
  MLP (gelu-tanh), residual -> outT.

Matmuls run in bf16 (tolerance 2e-2 >> bf16 error).
"""

import numpy as np
import ml_dtypes
from contextlib import ExitStack

import concourse.bass as bass
import concourse.bacc as bacc
import concourse.tile as tile
from concourse import mybir
from concourse import bass_utils

B, S, E, H, D = 4, 2048, 1024, 16, 64
HID = 4 * E
T = 1024            # query tokens per core
SKV = 1408          # gathered kv tokens per core (11 chunks of 128)
NCH = SKV // 128    # 11 s-chunks
NE = E // 128       # 8 feature chunks
NHT = HID // 128    # 32 hidden chunks
N_CORES = 8
# Active s-chunk pairs per query group. For queries q < 512 (qg0), keys are
# causally <= 511 (chunks 0-3) plus gathered summary chunks 8-10; chunks 4-7
# are fully masked for every core (host asserts this).
PAIRS_QG = [
    [(0, 1), (2, 3), (8, 9), (10,)],
    [(0, 1), (2, 3), (4, 5), (6, 7), (8, 9), (10,)],
]
MASK_J0 = [0, len(PAIRS_QG[0])]           # mask tile base index per qg
N_MASK = len(PAIRS_QG[0]) + len(PAIRS_QG[1])
MASK_W = 1024
QGS = [slice(0, 512), slice(512, 1024)]
KVGS = [slice(0, 512), slice(512, 1024), slice(1024, 1408)]
BF = mybir.dt.bfloat16
F32 = mybir.dt.float32
AF = mybir.ActivationFunctionType
OP = mybir.AluOpType

_CACHE = {}
import os
SIM_SAFE_GELU = os.environ.get("BASS_SIM_SAFE_GELU") == "1"


# ---------------------------------------------------------------- device code

def _tln(tc, ctx, src, dst, g_ap, b_ap, W, groups, ones, eps_row, tag,
         ones_f32=None):
    """Transposed layernorm: src [128, NE, W] bf16 -> dst [128, NE, W] bf16.

    Stats are partition reductions via ones-matmuls; mean/rstd rows get
    partition-broadcast via rank-1 PE matmuls (ones_f32 outer product) into
    PSUM -- no DRAM bounce. g_ap/b_ap: [128, NE]."""
    nc = tc.nc
    tmp = ctx.enter_context(tc.tile_pool(name=f"tmp{tag}", bufs=2))
    rows = ctx.enter_context(tc.tile_pool(name=f"rows{tag}", bufs=1))
    bc = ctx.enter_context(tc.tile_pool(name=f"bc{tag}", bufs=1, space="PSUM"))

    # stats rows live in partition 0 of the (later) broadcast tiles: the
    # row chain consumes them before the rank-1 matmuls overwrite the banks
    mean_b = bc.tile([128, W], F32, tag="mean_b")
    rstd_b = bc.tile([128, W], F32, tag="rstd_b")
    ps_mean = mean_b[0:1, :]
    ps_sq = rstd_b[0:1, :]
    for gs in groups:
        for c in range(NE):
            nc.tensor.matmul(ps_mean[:, gs], ones[:], src[:, c, gs],
                             start=(c == 0), stop=(c == NE - 1),
                             skip_group_check=True)
    for c in range(NE):
        sq = tmp.tile([128, W], BF, tag="sq")
        nc.scalar.activation(sq[:], src[:, c, :], AF.Square)
        for gs in groups:
            nc.tensor.matmul(ps_sq[:, gs], ones[:], sq[:, gs],
                             start=(c == 0), stop=(c == NE - 1),
                             skip_group_check=True)

    _tln_finish(tc, ctx, src, dst, g_ap, b_ap, groups, ones_f32, mean_b,
                rstd_b, eps_row, tag)


def _tln_finish(tc, ctx, src, dst, g_ap, b_ap, groups, ones_f32, mean_b,
                rstd_b, eps_row, tag):
    """LN tail: raw sums sit in partition 0 of mean_b/rstd_b (PSUM).
    Per token-group: compute mean/rstd rows, rank-1 partition-broadcast
    them back into the same banks, normalize src -> dst. Group-at-a-time
    so downstream consumers of dst[:, :, g] unblock before later groups."""
    nc = tc.nc
    tmp = ctx.enter_context(tc.tile_pool(name=f"tmpf{tag}", bufs=4))
    rows = ctx.enter_context(tc.tile_pool(name=f"rows{tag}", bufs=2))
    ps_mean = mean_b[0:1, :]
    ps_sq = rstd_b[0:1, :]

    for gs in groups:
        n = gs.stop - gs.start
        mean_r = rows.tile([1, 512], F32, tag="rowA")
        nc.scalar.mul(mean_r[:, :n], ps_mean[:, gs], 1.0 / E)
        msq_r = rows.tile([1, 512], F32, tag="rowB")
        nc.vector.tensor_mul(msq_r[:, :n], mean_r[:, :n], mean_r[:, :n])
        var_r = rows.tile([1, 512], F32, tag="rowC")
        nc.vector.scalar_tensor_tensor(var_r[:, :n], ps_sq[:, gs], 1.0 / E,
                                       msq_r[:, :n],
                                       op0=OP.mult, op1=OP.subtract)
        std_r = rows.tile([1, 512], F32, tag="rowD")
        nc.scalar.activation(std_r[:, :n], var_r[:, :n], AF.Sqrt,
                             bias=eps_row[:])
        rstd_r = rows.tile([1, 512], F32, tag="rowE")
        # approx reciprocal (~5x faster): this single-partition row op sits
        # on the serial LN critical path; 51 ULP is ample for rstd
        nc.vector.reciprocal_approx_fast(out=rstd_r[:, :n], in_=std_r[:, :n])

        # partition-broadcast the two rows on PE: out[128, n] =
        # ones[1,128].T @ row[1, n] (rank-1) into the group's PSUM bank
        nc.tensor.matmul(mean_b[:, gs], ones_f32[:], mean_r[0:1, :n],
                         start=True, stop=True, skip_group_check=True)
        nc.tensor.matmul(rstd_b[:, gs], ones_f32[:], rstd_r[0:1, :n],
                         start=True, stop=True, skip_group_check=True)

        for c in range(NE):
            t0 = tmp.tile([128, 512], F32, tag="t0")
            nc.vector.tensor_sub(t0[:, :n], src[:, c, gs], mean_b[:, gs])
            nc.vector.tensor_mul(t0[:, :n], t0[:, :n], rstd_b[:, gs])
            # gain/bias pass on ACT (idle here) instead of DVE: Identity
            # activation computes in*scale + bias with per-partition APs
            nc.scalar.activation(dst[:, c, gs], t0[:, :n], AF.Identity,
                                 bias=b_ap[:, c:c + 1],
                                 scale=g_ap[:, c:c + 1])


def _program(tc, ctx, outT, ins):
    nc = tc.nc
    (xkv, wqs, wks, wvs, wos, wfcs, wprjs, mask_d, gb, bfc_d, bprj_d) = ins

    # long-lived tiles first (pool releases must be LIFO); x loads issue
    # before every other DMA so LN1 starts as early as possible
    h2_pool = ctx.enter_context(tc.tile_pool(name="h2T", bufs=1))
    h2T = h2_pool.tile([128, NE, T], BF)
    g2_pool = ctx.enter_context(tc.tile_pool(name="g2T", bufs=1))
    g2T = g2_pool.tile([128, NE, T], BF)

    const = ctx.enter_context(tc.tile_pool(name="const", bufs=1))

    hT_pool = tc.alloc_tile_pool(name="hT", bufs=1)
    hT = hT_pool.tile([128, NE, SKV], BF)
    for c in range(NE):
        nc.sync.dma_start(hT[:, c, :], xkv[c])

    ones = const.tile([128, 1], BF)
    nc.vector.memset(ones[:], 1.0)
    ones_f32 = const.tile([1, 128], F32)
    nc.vector.memset(ones_f32[:], 1.0)
    eps_row = const.tile([1, 1], F32)
    nc.vector.memset(eps_row[:], 1e-5)
    warm = const.tile([1, 1], F32)
    # gb cols: 0:8 ln1_g, 8:16 ln1_b, 16:24 ln2_g, 24:32 ln2_b,
    #          32:40 bq, 40:48 bk, 48:56 (unused), 56:64 bo + wo@bv
    gb_sb = const.tile([128, 64], F32)
    nc.sync.dma_start(gb_sb[:], gb[:])
    bfc_sb = const.tile([128, 32], F32)
    nc.sync.dma_start(bfc_sb[:], bfc_d[:])
    bprj_sb = const.tile([128, 8], F32)
    nc.sync.dma_start(bprj_sb[:], bprj_d[:])

    mask_pool = tc.alloc_tile_pool(name="maskp", bufs=1)
    mask_sb = mask_pool.tile([128, N_MASK, MASK_W], BF)

    # ---- Phase 1: LN1 in place: x (loaded into hT) -> hT ---------------
    with ExitStack() as p1:
        _tln(tc, p1, hT, hT, gb_sb[:, 0:8], gb_sb[:, 8:16], SKV, KVGS,
             ones, eps_row, "1", ones_f32=ones_f32)
    # preload the exp table now: ACT is idle through QKV, and nothing
    # evicts exp before attention ends (sqrt/square only return at LN2).
    # loading it lazily at the QKV->attention transition idles the PE
    # >3.4us and can latch the HAM clock gate at 4/8 for the whole phase
    nc.scalar.activation(warm[:], eps_row[:], AF.Exp)

    # ---- Phase 2: QKV projections --------------------------------------
    kqv = tc.alloc_tile_pool(name="kqv", bufs=1)
    kT = kqv.tile([128, NE, SKV], BF)
    qT = kqv.tile([128, NE, T], BF)
    # V' per head: 64 v columns + 64 ones columns; the AV matmul then
    # leaves the softmax denominator replicated on psum partitions 64:128
    v_sb = kqv.tile([128, NCH, H * 128], BF)
    v4 = v_sb.rearrange("p t (h d) -> p t h d", d=128)
    nc.vector.memset(v4[:, :, :, 64:128], 1.0)

    ws2_pool = tc.alloc_tile_pool(name="ws2", bufs=8)
    attn_pool = tc.alloc_tile_pool(name="attnT", bufs=1)
    attnT = attn_pool.tile([128, NE, T], BF)
    wv_pool = tc.alloc_tile_pool(name="wvall", bufs=1)
    wv_all = wv_pool.tile([128, 16, 512], BF)

    with ExitStack() as p2:
        pj = p2.enter_context(tc.tile_pool(name="pj2", bufs=4, space="PSUM"))

        # group-OUTER over tokens: group g's 64-matmul block only needs
        # LN1's group g, so the PE consumes token groups as LN1 finishes
        # them instead of stalling mid-chain on a later group
        def proj(wdram, dst, gsl, bcol):
            wts = []
            for eo in range(NE):
                wt = ws2_pool.tile([128, NE, 128], BF, tag="w")
                nc.sync.dma_start(wt[:], wdram[eo])
                wts.append(wt)
            for gs in gsl:
                n = gs.stop - gs.start
                for eo in range(NE):
                    ps = pj.tile([128, 512], F32, tag="ps")
                    for c in range(NE):
                        nc.tensor.matmul(ps[:, :n], wts[eo][:, c, :],
                                         hT[:, c, gs],
                                         start=(c == 0), stop=(c == NE - 1),
                                         skip_group_check=True)
                    nc.vector.tensor_scalar_add(
                        out=dst[:, eo, gs], in0=ps[:, :n],
                        scalar1=gb_sb[:, bcol + eo:bcol + eo + 1])

        proj(wks, kT, KVGS, 40)
        # wv load emitted after the k-weight loads so it doesn't block them
        # on the DMA queue (V matmuls run last; the 4MB load overlaps K/Q)
        for g2 in range(2):
            for c in range(NE):
                nc.sync.dma_start(wv_all[:, g2 * NE + c, :], wvs[g2, c])
        proj(wqs, qT, QGS, 32)
        # masks: needed only at attention start
        for j in range(N_MASK):
            nc.sync.dma_start(mask_sb[:, j, :], mask_d[j])
        # V token-major; e_out groups of 512 = 8 heads each
        for tt in range(NCH):
            for g2 in range(2):
                ps = pj.tile([128, 512], F32, tag="ps")
                for c in range(NE):
                    nc.tensor.matmul(ps[:], hT[:, c, tt * 128:(tt + 1) * 128],
                                     wv_all[:, g2 * NE + c, :],
                                     start=(c == 0), stop=(c == NE - 1),
                                     skip_group_check=True)
                nc.vector.tensor_copy(
                    out=v4[:, tt, g2 * 8:(g2 + 1) * 8, 0:64],
                    in_=ps[:].rearrange("p (h d) -> p h d", d=64))

    # ---- Phase 3: attention ---------------------------------------------
    wv_pool.release()

    with ExitStack() as p3:
        pss = p3.enter_context(tc.tile_pool(name="psS", bufs=2, space="PSUM"))
        pso = p3.enter_context(tc.tile_pool(name="psO", bufs=4, space="PSUM"))
        ptp = p3.enter_context(tc.tile_pool(name="pT", bufs=4))
        rcp = p3.enter_context(tc.tile_pool(name="rcp", bufs=3))

        # h outer / qg inner: interleaving the sparse-qg0 and dense-qg1
        # PE work keeps the tensor engine busy enough that the HAM clock
        # gate stays at 8/8 through the whole attention phase
        for h in range(H):
            pt, po = h // 2, (h % 2) * 64
            for qg in range(2):
                qs = QGS[qg]
                pairs = PAIRS_QG[qg]
                npair = len(pairs)
                ps_o = pso.tile([128, 512], F32, tag="o")
                for pi, pair in enumerate(pairs):
                    ps_s = pss.tile([128, MASK_W], F32, tag="s")
                    for k, c in enumerate(pair):
                        nc.tensor.matmul(
                            ps_s[:, k * 512:k * 512 + 512],
                            kT[po:po + 64, pt, c * 128:(c + 1) * 128],
                            qT[po:po + 64, pt, qs],
                            start=True, stop=True, skip_group_check=True)
                    w = len(pair) * 512
                    pT = ptp.tile([128, MASK_W], BF, tag="pT")
                    nc.scalar.activation(pT[:, :w], ps_s[:, :w], AF.Exp)
                    nc.vector.tensor_mul(pT[:, :w], pT[:, :w],
                                         mask_sb[:, MASK_J0[qg] + pi, :w])
                    for k, c in enumerate(pair):
                        nc.tensor.matmul(
                            ps_o[:, :],
                            v_sb[:, c, h * 128:h * 128 + 128],
                            pT[:, k * 512:k * 512 + 512],
                            start=(pi == 0 and k == 0),
                            stop=(pi == npair - 1 and k == len(pair) - 1),
                            skip_group_check=True)
                # denominator sits replicated on psum partitions 64:128
                # (ones half of V'): copy to SBUF, approx-reciprocal in
                # place (custom-DVE op needs SBUF input), multiply.
                # bv is host-folded into bo, so no bias add here.
                denb = rcp.tile([64, 512], F32, tag="denb")
                nc.vector.tensor_copy(denb[:], ps_o[64:128, :])
                nc.vector.reciprocal_approx_fast(out=denb[:], in_=denb[:])
                nc.vector.tensor_mul(attnT[po:po + 64, pt, qs],
                                     ps_o[0:64, :], denb[:])

    # ---- Phase 4: Wo + residual -> h2T; LN2 stats fused into the Wo
    # loop (each h2T chunk feeds the LN2 partition-reduce immediately,
    # interleaving stats matmuls with Wo matmuls on the PE)
    with ExitStack() as p45:
        bc2 = p45.enter_context(tc.tile_pool(name="bc2", bufs=1,
                                             space="PSUM"))
        mean_b2 = bc2.tile([128, T], F32, tag="mean_b")
        rstd_b2 = bc2.tile([128, T], F32, tag="rstd_b")
        with ExitStack() as p4:
            pj = p4.enter_context(tc.tile_pool(name="pj4", bufs=4,
                                               space="PSUM"))
            sqp = p4.enter_context(tc.tile_pool(name="sq4", bufs=2))
            for eo in range(NE):
                wt = ws2_pool.tile([128, NE, 128], BF, tag="w")
                nc.sync.dma_start(wt[:], wos[eo])
                for qg in range(2):
                    qs = QGS[qg]
                    ps = pj.tile([128, 512], F32, tag="ps")
                    for c in range(NE):
                        nc.tensor.matmul(ps[:], wt[:, c, :], attnT[:, c, qs],
                                         start=(c == 0), stop=(c == NE - 1),
                                         skip_group_check=True)
                    nc.vector.scalar_tensor_tensor(
                        h2T[:, eo, qs], ps[:], gb_sb[:, 56 + eo:56 + eo + 1],
                        hT[:, eo, qs], op0=OP.add, op1=OP.add)
                sq = sqp.tile([128, T], BF, tag="sq")
                nc.scalar.activation(sq[:], h2T[:, eo, :], AF.Square)
                for gs in QGS:
                    nc.tensor.matmul(mean_b2[0:1, gs], ones[:],
                                     h2T[:, eo, gs], start=(eo == 0),
                                     stop=(eo == NE - 1),
                                     skip_group_check=True)
                    nc.tensor.matmul(rstd_b2[0:1, gs], ones[:], sq[:, gs],
                                     start=(eo == 0), stop=(eo == NE - 1),
                                     skip_group_check=True)

        attn_pool.release()
        ws2_pool.release()
        kqv.release()
        mask_pool.release()
        hT_pool.release()

        with ExitStack() as p5:
            _tln_finish(tc, p5, h2T, g2T, gb_sb[:, 16:24], gb_sb[:, 24:32],
                        QGS, ones_f32, mean_b2, rstd_b2, eps_row, "2")

    # ---- Phase 5: MLP + residual -> outT -------------------------------
    with ExitStack() as p6:
        ws = p6.enter_context(tc.tile_pool(name="ws6", bufs=8))
        wp = p6.enter_context(tc.tile_pool(name="wp6", bufs=3))
        pj = p6.enter_context(tc.tile_pool(name="pj6", bufs=4, space="PSUM"))
        up = p6.enter_context(tc.tile_pool(name="uT", bufs=1))
        op_ = p6.enter_context(tc.tile_pool(name="outp", bufs=3))
        uT = up.tile([128, NHT, T], BF, tag="uT")
        for ht in range(NHT):
            wt = ws.tile([128, NE, 128], BF, tag="w")
            nc.sync.dma_start(wt[:], wfcs[ht])
            for qg in range(2):
                qs = QGS[qg]
                ps = pj.tile([128, 512], F32, tag="ps")
                for c in range(NE):
                    nc.tensor.matmul(ps[:], wt[:, c, :], g2T[:, c, qs],
                                     start=(c == 0), stop=(c == NE - 1),
                                     skip_group_check=True)
                if not SIM_SAFE_GELU:
                    nc.scalar.activation(uT[:, ht, qs], ps[:],
                                         AF.Gelu_apprx_tanh,
                                         bias=bfc_sb[:, ht:ht + 1])
                else:
                    # new_gelu(u) = u * sigmoid(2c*(u + 0.044715 u^3))
                    u_sb = ws.tile([128, 512], BF, tag="gelu_u")
                    nc.vector.tensor_scalar_add(
                        out=u_sb[:], in0=ps[:],
                        scalar1=bfc_sb[:, ht:ht + 1])
                    t = ws.tile([128, 512], BF, tag="gelu_t")
                    nc.scalar.activation(t[:], u_sb[:], AF.Square)
                    z = ws.tile([128, 512], BF, tag="gelu_z")
                    nc.vector.scalar_tensor_tensor(
                        z[:], t[:], 0.044715, u_sb[:],
                        op0=OP.mult, op1=OP.mult)
                    nc.vector.tensor_add(z[:], z[:], u_sb[:])
                    g = ws.tile([128, 512], BF, tag="gelu_g")
                    nc.scalar.activation(
                        g[:], z[:], AF.Sigmoid,
                        scale=float(2.0 * np.sqrt(2.0 / np.pi)))
                    nc.vector.tensor_mul(uT[:, ht, qs], u_sb[:], g[:])
        for eo in range(NE):
            wt = wp.tile([128, NHT, 128], BF, tag="w")
            nc.sync.dma_start(wt[:], wprjs[eo])
            for qg in range(2):
                qs = QGS[qg]
                ps = pj.tile([128, 512], F32, tag="ps")
                for c in range(NHT):
                    nc.tensor.matmul(ps[:], wt[:, c, :], uT[:, c, qs],
                                     start=(c == 0), stop=(c == NHT - 1),
                                     skip_group_check=True)
                ot = op_.tile([128, 512], F32, tag="ot")
                nc.vector.scalar_tensor_tensor(
                    ot[:], ps[:], bprj_sb[:, eo:eo + 1], h2T[:, eo, qs],
                    op0=OP.add, op1=OP.add)
                nc.sync.dma_start(outT[eo][:, qs], ot[:])


def _build():
    if "nc" in _CACHE:
        return _CACHE["nc"]
    nc = bacc.Bacc("TRN2", target_bir_lowering=False, debug=False,
                   num_devices=N_CORES)

    def din(name, shape, dt=BF):
        return nc.dram_tensor(name, shape, dt, kind="ExternalInput").ap()

    ins = [
        din("xkv", [NE, 128, SKV]),
        din("wqs", [NE, 128, NE, 128]),
        din("wks", [NE, 128, NE, 128]),
        din("wvs", [2, NE, 128, 512]),
        din("wos", [NE, 128, NE, 128]),
        din("wfcs", [NHT, 128, NE, 128]),
        din("wprjs", [NE, 128, NHT, 128]),
        din("mask", [N_MASK, 128, MASK_W]),
        din("gb", [128, 64], F32),
        din("bfc", [128, 32], F32),
        din("bprj", [128, 8], F32),
    ]
    outT = nc.dram_tensor("outT", [NE, 128, T], F32, kind="ExternalOutput").ap()

    with tile.TileContext(nc) as tc:
        with ExitStack() as ctx:
            _program(tc, ctx, outT, ins)
    nc.compile()
    _CACHE["nc"] = nc
    return nc


# ------------------------------------------------------------------ host code

def _prep_shared(args):
    (ln1_g, ln1_b, ln2_g, ln2_b, wq, bq, wk, bk, wv, bv, wo, bo,
     w_fc, b_fc, w_proj, b_proj) = [np.asarray(a, np.float32) for a in args]
    bf = ml_dtypes.bfloat16

    def eo_layout(wT, nk, nm):  # wT [nk*128, nm*128] -> [nm, 128, nk, 128]
        return np.ascontiguousarray(
            wT.reshape(nk, 128, nm, 128).transpose(2, 1, 0, 3).astype(bf))

    def cols(v, n):  # [n*128] -> [128, n]
        return np.ascontiguousarray(
            np.asarray(v, np.float32).reshape(n, 128).T)

    scale = np.float32(D ** -0.5)
    d = {}
    d["wqs"] = eo_layout((wq * scale).T, NE, NE)
    d["wks"] = eo_layout(wk.T, NE, NE)
    d["wvs"] = np.ascontiguousarray(
        wv.T.reshape(NE, 128, 2, 512).transpose(2, 0, 1, 3).astype(bf))
    d["wos"] = eo_layout(wo.T, NE, NE)
    d["wfcs"] = eo_layout(w_fc.T, NE, NHT)
    d["wprjs"] = eo_layout(w_proj.T, NHT, NE)

    gb = np.zeros((128, 64), np.float32)
    gb[:, 0:8] = cols(ln1_g, 8)
    gb[:, 8:16] = cols(ln1_b, 8)
    gb[:, 16:24] = cols(ln2_g, 8)
    gb[:, 24:32] = cols(ln2_b, 8)
    gb[:, 32:40] = cols(bq * scale, 8)
    gb[:, 40:48] = cols(bk, 8)
    # bv folded into bo: attn_out = (softmax(..)V + bv) Wo^T + bo
    #                             = softmax(..)V Wo^T + (bo + Wo bv)
    gb[:, 56:64] = cols(bo + wo @ bv, 8)
    d["gb"] = gb
    d["bfc"] = cols(b_fc, 32)
    d["bprj"] = cols(b_proj, 8)
    return d


def _core_inputs(x, mask, shared, core):
    bf = ml_dtypes.bfloat16
    b, half = divmod(core, 2)
    qtok = np.arange(half * T, (half + 1) * T)
    need = np.where(mask[qtok].any(axis=0))[0]
    extra = np.setdiff1d(need, qtok)
    nreal = T + len(extra)
    assert nreal <= SKV, (core, nreal)
    kv = np.concatenate([qtok, extra,
                         np.zeros(SKV - nreal, np.int64)])

    xkvT = x[b].T[:, kv].astype(bf)  # [E, SKV]
    m = np.zeros((T, SKV), np.float32)
    m[:, :nreal] = mask[np.ix_(qtok, kv[:nreal])]
    mT = m.T  # [SKV, T]
    mtiles = np.zeros((N_MASK, 128, MASK_W), np.float32)
    for qg in range(2):
        active = {c for pair in PAIRS_QG[qg] for c in pair}
        skipped = [c for c in range(NCH) if c not in active]
        for c in skipped:
            assert not mT[c * 128:(c + 1) * 128,
                          qg * 512:(qg + 1) * 512].any(), (core, qg, c)
        for pi, pair in enumerate(PAIRS_QG[qg]):
            for k, c in enumerate(pair):
                mtiles[MASK_J0[qg] + pi, :, k * 512:(k + 1) * 512] = \
                    mT[c * 128:(c + 1) * 128, qg * 512:(qg + 1) * 512]

    im = dict(shared)
    im["xkv"] = np.ascontiguousarray(xkvT.reshape(NE, 128, SKV))
    im["mask"] = np.ascontiguousarray(mtiles.astype(bf))
    return im, qtok


def kernel(x, ln1_g, ln1_b, ln2_g, ln2_b, wq, bq, wk, bk, wv, bv, wo, bo,
           w_fc, b_fc, w_proj, b_proj, mask):
    x = np.asarray(x, np.float32)
    mask = np.asarray(mask, bool)

    nc = _build()
    shared = _prep_shared((ln1_g, ln1_b, ln2_g, ln2_b, wq, bq, wk, bk, wv,
                           bv, wo, bo, w_fc, b_fc, w_proj, b_proj))

    in_maps, qtoks = [], []
    for core in range(N_CORES):
        im, qtok = _core_inputs(x, mask, shared, core)
        in_maps.append(im)
        qtoks.append(qtok)

    br = bass_utils.run_bass_kernel_spmd(nc, in_maps,
                                         core_ids=list(range(N_CORES)))
    out = np.empty((B, S, E), np.float32)
    for core in range(N_CORES):
        b, half = divmod(core, 2)
        oT = br.results[core]["outT"]  # [NE, 128, T]
        out[b, qtoks[core], :] = oT.reshape(E, T).T
    return out

